# revision 131
# baseline (speedup 1.0000x reference)
"""nn_AttnA: fused QKV-proj + RMSnorm + RoPE + causal GQA attention + out-proj.

Data-parallel over the batch: core b computes batch element b (B=8 = 8 cores,
no collectives). Host pre-transposes/casts weights and x once; the device
kernel is fully self-contained per core.

Device pipeline per core (T=2048, C=512, 8 q-heads / 4 kv-heads, hd=64):
  1. QKV: fp16 matmuls, xT c-tiles stationary, fused [q|k|v] rhs -> a psum
     slot of the shared sc rotation ([128,1024] f32 x2)
  2. RMS stats + rstd (ACT Ln/Exp from table set 6, loaded once) + RoPE on
     DVE; v gets a 65th all-ones column per kv-head so attnV emits softmax
     denominators for free
  3. PE transposes -> qT [d,t] per head pair; kT duplicated into both row
     halves so the pair's score matmuls row-pack (concurrent K=64 strips)
  4. per (head-pair, 512-wide q chunk): both heads' score matmuls fill one
     [128,1024] sc slot; ONE merged ACT Exp (scale=1/8) -> fp16 pT
     [128,1024]; one 3D-strided triangle mask on diagonal blocks; FLIPPED
     attnV: each 128-wide pT q-block is the STATIONARY operand and the
     65-wide v the moving one, so each matmul costs only 65 output columns
     (29.5us PE total instead of 58us) and the softmax denominator lands
     per-PARTITION at out col 64 of bank-sized [128, 4, 128] f32
     accumulators (start=True only on the bank's first matmul: a start
     zeroes a whole 2KB ZERO_REGION). Normalize is then 2 tiny reciprocals
     + 2 per-partition muls -> y [q,d] fp16, transposed back to yT [d,t] by
     8 PE transposes through the op bank. No cross-partition broadcast, no
     partition-shift DMA.
  5. out-proj: yT t-slices stationary x WpT -> [t, o] fp32 -> DRAM

Emission is software-pipelined: prep A runs one chunk ahead of prep B;
attnV trails the score/exp stream by 4-5 k-steps; each chunk's ENTIRE tail
(trailing attnV drain at k=0/1, reciprocals at k=2, normalize muls at k=3,
transposes+writeback at k=5) is threaded into the next chunk's k-loop so
the in-order PE issues next-chunk scores right at the boundary; prep fills
go one-per-k-step from k=3 (qkv psum release via ACT copies for tau<9,
DVE for later taus, matching which engine has queue slack); out-proj fills
wait until k>=8 (their yT is written by the deferred tail). The final-row
out-projs pre-accumulate pairs 0-2 inside the last chunk.

Constraints learned on HW: GPSIMD/Pool cannot touch PSUM; TensorTensor may
read at most ONE input from PSUM; psum start=True zeroes 2KB regions; fp8
(e4m3) DoubleRow fails the 2e-2 gate everywhere (3-6e-2 measured).

Cost-model timeline: 221us (prev session 253us, stub 379us), verified on
HW at rel err 4.6e-4. Engine busy: ACT ~168us (exp floor: 139k softmax
cols at 1.2GHz + 185ns/instr), PE ~149us, DVE ~114us.
"""
import numpy as np
from contextlib import ExitStack

import concourse.bacc as bacc
import concourse.bass as bass
import concourse.tile as tile
from concourse import mybir
from concourse.bass_utils import run_bass_kernel_spmd
from concourse.masks import make_identity

F32 = mybir.dt.float32
F16 = mybir.dt.float16
AF = mybir.ActivationFunctionType

DIM = 512
EPS = 1.1920928955078125e-07
SCALE = 0.125  # 1/sqrt(64)
ROPE_BASE = 10000.0
N_CORES = 8
ACT_SET_LN_EXP = 6  # natural_log_exp_and_others: serves ln + exp + copy


def build_kernel(T=2048, reps=1, variant="full"):
    """reps>1 re-emits the compute body for delta-timing benchmarks."""
    P = 128
    TT = T // 128
    QC = T // 512
    NPAIR = 4
    VW = 65  # v columns per kv-head incl the ones column

    nc = bacc.Bacc()
    xT = nc.declare_dram_parameter("xT", [DIM, T], F16, isOutput=False)
    wqkvT = nc.declare_dram_parameter("wqkvT", [DIM, 1024], F16, isOutput=False)
    wpT = nc.declare_dram_parameter("wpT", [DIM, DIM], F16, isOutput=False)
    cosd = nc.declare_dram_parameter("cosd", [T, 32], F16, isOutput=False)
    sind = nc.declare_dram_parameter("sind", [T, 32], F16, isOutput=False)
    trid = nc.declare_dram_parameter("trid", [P, P], F16, isOutput=False)
    out = nc.declare_dram_parameter("out", [T, DIM], F32, isOutput=True)

    with tile.TileContext(nc) as tc, ExitStack() as ctx:
        consts = ctx.enter_context(tc.tile_pool(name="consts", bufs=1))
        big = ctx.enter_context(tc.tile_pool(name="big", bufs=1))
        work = ctx.enter_context(tc.tile_pool(name="work", bufs=2))
        pT_pool = ctx.enter_context(tc.tile_pool(name="pT", bufs=2))
        outp = ctx.enter_context(tc.tile_pool(name="outp", bufs=4))
        psA = ctx.enter_context(tc.tile_pool(name="psA", bufs=1, space="PSUM"))
        psC = ctx.enter_context(tc.tile_pool(name="psC", bufs=1, space="PSUM"))

        # Single activation-table load serving Ln + Exp + Copy; without it the
        # auto-pass alternates set 5 (ln) / set 0 (exp) at 1283ns per load.
        nc.scalar.add_instruction(mybir.InstLoadActFuncSet(
            name=nc.get_next_instruction_name(),
            act_func_set_id=ACT_SET_LN_EXP, ins=[], outs=[]))

        ident = consts.tile([P, P], F16)
        make_identity(nc, ident)
        eps_b = consts.tile([P, 1], F32)
        nc.vector.memset(eps_b, EPS)
        tri = consts.tile([P, P], F16)
        cos_sb = consts.tile([P, TT * 32], F16)
        sin_sb = consts.tile([P, TT * 32], F16)

        xT_sb = big.tile([P, 4, T], F16)
        wqkv_sb = big.tile([P, 4, 1024], F16)
        wp_sb = big.tile([P, 4, DIM], F16)
        # Balance input loads across the two HW DGE queues (SP via nc.sync,
        # ACT via nc.scalar) and merge c-slices into single DMAs — each
        # dma_start costs >1.2us of sequencer issue time, which dominates
        # the prologue if the loads are issued one slice at a time.
        # The DMA transfers serialize on the DMA engine, so order by first
        # use: rope tables, then the xT columns the 5 prologue preps read,
        # then weights, then the rest of xT (consumed from tau 5 on, ~25us
        # in). Issue cost is >1.2us per dma_start, so slices are merged.
        FC = min(4 * P, T)  # xT columns needed by the prologue preps
        nc.sync.dma_start(
            out=xT_sb[:, :, 0:FC],
            in_=xT.rearrange("(c p) t -> p c t", p=P)[:, :, 0:FC])
        nc.scalar.dma_start(out=wqkv_sb[:, 0:2, :],
                            in_=wqkvT[0:2 * P, :].rearrange("(c p) t -> p c t", p=P))
        nc.sync.dma_start(out=wqkv_sb[:, 2:4, :],
                          in_=wqkvT[2 * P:4 * P, :].rearrange("(c p) t -> p c t", p=P))
        nc.scalar.dma_start(out=cos_sb.rearrange("p (tau i) -> p tau i", i=32),
                            in_=cosd.rearrange("(tau p) i -> p tau i", p=P))
        nc.sync.dma_start(out=sin_sb.rearrange("p (tau i) -> p tau i", i=32),
                          in_=sind.rearrange("(tau p) i -> p tau i", p=P))
        if FC < T:
            nc.scalar.dma_start(
                out=xT_sb[:, :, FC:T],
                in_=xT.rearrange("(c p) t -> p c t", p=P)[:, :, FC:T])
        nc.scalar.dma_start(out=tri, in_=trid[:, :])

        # PE p-state warm-up: the cost model runs the PE at half clock until
        # 3us of continuous busy; a transpose train during the input-DMA wait
        # ramps it to 2.4GHz before the first real matmul
        warm_ps = psC.tile([P, P], F16, tag="op", name="warm_ps")
        for _ in range(16):
            nc.tensor.transpose(warm_ps, ident, ident)
        qT_sb = big.tile([P, NPAIR * T], F16)
        kT_sb = big.tile([P, NPAIR * T], F16)
        v_sb = big.tile([P, TT, 4, VW], F16)
        yT_sb = big.tile([P, NPAIR * T], F16)
        # ones column (col 64 of each kv-head group), written once
        nc.vector.memset(v_sb[:, :, :, 64:65], 1.0)

        def prep_a(tau):
            """QKV matmuls + psum->sbuf copies + RMS stats + RoPE -> 'prep'.
            The qkv psum comes from the shared sc rotation."""
            qkv_ps = psA.tile([P, 1024], F32, tag="sc", bufs=2, name="qkv_ps")
            for c in range(4):
                lhs = xT_sb[:, c, tau * P:(tau + 1) * P]
                nc.tensor.matmul(qkv_ps[:, 0:512], lhs, wqkv_sb[:, c, 0:512],
                                 start=(c == 0), stop=(c == 3))
                nc.tensor.matmul(qkv_ps[:, 512:1024], lhs, wqkv_sb[:, c, 512:1024],
                                 start=(c == 0), stop=(c == 3))
            qk16 = work.tile([P, 768], F16, tag="qk16")
            if tau < 9:
                # early: ACT has slack and releases the qkv psum slot fast
                nc.scalar.activation(qk16, qkv_ps[:, 0:768], AF.Copy)
                nc.scalar.activation(
                    v_sb[:, tau, :, 0:64],
                    qkv_ps[:, 768:1024].rearrange("p (h d) -> p h d", d=64),
                    AF.Copy)
            else:
                # late: ACT is the exp-saturated bottleneck; DVE has slack
                # there (GPSIMD cannot touch PSUM on HW)
                nc.vector.tensor_copy(qk16, qkv_ps[:, 0:768])
                nc.vector.tensor_copy(
                    v_sb[:, tau, :, 0:64],
                    qkv_ps[:, 768:1024].rearrange("p (h d) -> p h d", d=64))
            sq16 = work.tile([P, 768], F16, tag="sq16")
            if tau <= 4:
                # DVE is the prep-chain rate limiter while preps overlap the
                # short early rows; ACT has slack there
                nc.scalar.activation(sq16, qk16, AF.Square)
            else:
                nc.vector.tensor_mul(sq16, qk16, qk16)
            ms = work.tile([P, 12], F16, tag="ms")
            with nc.allow_low_precision(reason="fp16 mean-of-squares"):
                nc.vector.tensor_reduce(ms, sq16.rearrange("p (h d) -> p h d", d=64),
                                        axis=mybir.AxisListType.X,
                                        op=mybir.AluOpType.add)
            lns = work.tile([P, 12], F32, tag="lns")
            nc.scalar.activation(lns, ms, AF.Ln, scale=1.0 / 64, bias=eps_b)
            r32 = work.tile([P, 12], F32, tag="r32")
            nc.scalar.activation(r32, lns, AF.Exp, scale=-0.5)
            qkr = work.tile([P, 768], F16, tag="qkr")
            nc.vector.tensor_mul(qkr.rearrange("p (h d) -> p h d", d=64),
                                 qk16.rearrange("p (h d) -> p h d", d=64),
                                 r32[:, :, None].broadcast_to([P, 12, 64]))
            qkrh = qkr.rearrange("p (h d) -> p h d", d=64)
            x1, x2 = qkrh[:, :, 0:32], qkrh[:, :, 32:64]
            c_b = cos_sb[:, tau * 32:(tau + 1) * 32][:, None, :].broadcast_to([P, 12, 32])
            s_b = sin_sb[:, tau * 32:(tau + 1) * 32][:, None, :].broadcast_to([P, 12, 32])
            t1 = work.tile([P, 12, 32], F16, tag="t1")
            t2 = work.tile([P, 12, 32], F16, tag="t2")
            t3 = work.tile([P, 12, 32], F16, tag="t3")
            t4 = work.tile([P, 12, 32], F16, tag="t4")
            nc.vector.tensor_mul(t1, x1, c_b)
            nc.vector.tensor_mul(t2, x2, s_b)
            nc.vector.tensor_mul(t3, x1, s_b)
            nc.vector.tensor_mul(t4, x2, c_b)
            prep = work.tile([P, 768], F16, tag="prep")
            ph = prep.rearrange("p (h d) -> p h d", d=64)
            nc.vector.tensor_add(ph[:, :, 0:32], t1, t2)
            nc.vector.tensor_sub(ph[:, :, 32:64], t4, t3)
            return prep

        def prep_b(tau, prep, tr_tag="pq"):
            """PE transposes of 'prep' + writeback into qT/kT column layout.
            q transposes fill cols 0:512, k (duplicated row halves) 512:1024
            of one [128,1024]-f16 psum bank. During the prologue the attn
            psum tags (psC pool) are free, so transposes rotate through them
            and the pq bank never serializes consecutive prep chains."""
            pool = psA if tr_tag == "pq" else psC
            trk_ps = pool.tile([P, 1024], F16, tag=tr_tag, bufs=1, name="trk_ps")
            for blk in range(4):
                nc.tensor.transpose(trk_ps[:, blk * P:(blk + 1) * P],
                                    prep[:, blk * P:(blk + 1) * P], ident)
            for kv in range(4):
                kin = prep[:, 512 + kv * 64: 512 + (kv + 1) * 64]
                nc.tensor.transpose(trk_ps[0:64, 512 + kv * P: 512 + (kv + 1) * P],
                                    kin, ident)
                nc.tensor.transpose(trk_ps[64:128, 512 + kv * P: 512 + (kv + 1) * P],
                                    kin, ident, tile_position=(0, 64))
            qdst = bass.AP(tensor=qT_sb.tensor, offset=qT_sb.offset + tau * P,
                           ap=[qT_sb.ap[0], [T, 4], [1, P]])
            kdst = bass.AP(tensor=kT_sb.tensor, offset=kT_sb.offset + tau * P,
                           ap=[kT_sb.ap[0], [T, 4], [1, P]])
            nc.vector.tensor_copy(qdst, trk_ps[:, 0:512].rearrange("p (g t) -> p g t", t=P))
            nc.vector.tensor_copy(kdst, trk_ps[:, 512:1024].rearrange("p (g t) -> p g t", t=P))

        def attn_pair_chunk(p, j, fills=(), late_fills=(),
                            prev_tail=(None, None), final=False,
                            mask_eng=None):
            """Emits one (head-pair, 512-q-chunk) of attention. Returns two
            tail closures (reciprocal; broadcast+normalize+shift) that the
            CALLER threads into the next chunk's k-loop — emitted at k=0/k=1
            there, they overlap the tail latency with the next chunk's score
            stream instead of stalling the in-order PE at the boundary."""
            nkt = 4 * j + 4
            fills = list(fills)
            late_fills = list(late_fills)
            # [q-part, block, d+den] accumulators, one per head (flipped
            # attnV: pT stationary, 65-wide v moving -> 65-col matmuls and
            # per-PARTITION softmax denominators at out col 64).
            # Block stride is 128 f32 so each tile is exactly one 2KB psum
            # bank: start=True zeroes a whole ZERO_REGION (2KB), so only
            # the very first matmul of each bank may carry it — the other
            # blocks' first writes land on pending-zero bytes (write mode).
            yq_e = psC.tile([P, 4, P], F32, tag="yTe", name="yq_e")
            yq_o = psC.tile([P, 4, P], F32, tag="yTo", name="yq_o")
            pTs = {}

            def attn_v(k):
                pT = pTs.pop(k)
                v65 = v_sb[:, k, p, :]
                for b in range(4):
                    if 4 * j + b < k:
                        continue  # q-block entirely above the diagonal
                    st = (k == 0 and b == 0)
                    sp = (k == 4 * j + b)
                    nc.tensor.matmul(yq_e[:, b, 0:VW], pT[:, b * P:(b + 1) * P],
                                     v65, start=st, stop=sp,
                                     skip_group_check=not st)
                    nc.tensor.matmul(yq_o[:, b, 0:VW],
                                     pT[:, 512 + b * P:512 + (b + 1) * P],
                                     v65, start=st, stop=sp,
                                     skip_group_check=not st)

            for k in range(nkt):
                offs = max(0, P * (k - 4 * j))
                kcol = p * T + k * P
                qcol = p * T + 512 * j + offs
                n = 512 - offs
                # both heads' scores fill one [128,1024] slot; 2-deep
                # rotation lets scores-mm(k+1) overlap exp(k)
                sc = psA.tile([P, 1024], F32, tag="sc", bufs=2, name="sc")
                nc.tensor.matmul(sc[:, offs:512],
                                 kT_sb[0:64, kcol:kcol + P],
                                 qT_sb[0:64, qcol:qcol + n],
                                 start=True, stop=True)
                nc.tensor.matmul(sc[:, 512 + offs:1024],
                                 kT_sb[64:128, kcol:kcol + P],
                                 qT_sb[64:128, qcol:qcol + n],
                                 start=True, stop=True, tile_position=(64, 0))
                pT = pT_pool.tile([P, 1024], F16, tag="pT", bufs=8)
                pTs[k] = pT
                sch = sc.rearrange("p (h n) -> p h n", n=512)
                pTh = pT.rearrange("p (h n) -> p h n", n=512)
                # ONE merged exp for both heads (3D strided AP)
                nc.scalar.activation(pTh[:, :, offs:512], sch[:, :, offs:512],
                                     AF.Exp, scale=SCALE)
                if k >= 4 * j:  # diagonal tile: mask strict lower triangle
                    # on the (otherwise idle) Pool engine — SBUF-only op, so
                    # it is legal there and keeps the chain off the DVE queue
                    sl = slice(offs, offs + P)
                    (mask_eng or nc.vector).tensor_mul(
                        pTh[:, :, sl], pTh[:, :, sl],
                        tri[:, None, :].broadcast_to([P, 2, P]))
                # Thread the PREVIOUS chunk's trailing work (attnV drain,
                # recips, normalize) into this chunk's score stream: the
                # in-order PE then issues this chunk's scores right at the
                # boundary instead of serially waiting exp->mask->attnV->
                # normalize of the previous chunk (was a ~2.9us ACT gap
                # per chunk).
                if prev_tail[0] is not None:
                    pdrains, pt0, pt1a, pt1b = prev_tail
                    nd = len(pdrains)
                    if k == 0:
                        for d in pdrains[:nd // 2]:
                            d()
                    elif k == 1:
                        for d in pdrains[nd // 2:]:
                            d()
                    elif k == 2:
                        pt0()
                    elif k == 3:
                        pt1a()
                    elif k == min(5, nkt - 1):
                        pt1b()
                # attnV trails the score/exp stream so the PE never blocks
                # on the exp of the current k
                dly = 4 if nkt <= 8 else 5
                if k >= dly:
                    attn_v(k - dly)
                # Prep emissions fill from k=3, one per k-step so the burst
                # never outruns the 2-deep sc pipeline and starves ACT.
                # Out-proj fills (late_fills) must wait for the k=5 thread
                # point: they read yT columns that the previous chunk's
                # deferred tail1b writes there.
                if k >= 3 and fills:
                    fills.pop(0)()
                elif k >= 9 and late_fills:
                    late_fills.pop(0)()
            if prev_tail[0] is not None and nkt <= 4:
                # nkt=4 chunks: the k=5 thread point doesn't exist
                pt1b()
            for f in fills + late_fills:
                if f is not None:
                    f()
            drains = [lambda k=k: attn_v(k)
                      for k in range(max(0, nkt - dly), nkt)]

            def tail0():
                # per-partition denominators at out col 64 -> tiny recips
                # (divide-from-psum is illegal: only one PSUM input allowed)
                rd = outp.tile([P, 8], F32, tag="rd")
                nc.vector.reciprocal(rd[:, 0:4], yq_e[:, :, 64])
                nc.vector.reciprocal(rd[:, 4:8], yq_o[:, :, 64])
                tail0.rd = rd

            def tail1a():
                rd = tail0.rd
                yn = outp.tile([P, 8, 64], F16, tag="yn")
                nc.vector.tensor_mul(
                    yn[:, 0:4, :], yq_e[:, :, 0:64],
                    rd[:, 0:4][:, :, None].broadcast_to([P, 4, 64]))
                nc.vector.tensor_mul(
                    yn[:, 4:8, :], yq_o[:, :, 0:64],
                    rd[:, 4:8][:, :, None].broadcast_to([P, 4, 64]))
                tail1a.yn = yn

            def tail1b():
                yn = tail1a.yn
                # transpose y [q,d] back to yT [d,q]; odd head packs into
                # partitions 64..127 via tile_position. Runs 2 k-steps after
                # tail1a so the PE never waits on the DVE normalize muls.
                ytp = psC.tile([P, 512], F16, tag="op", name="ytp")
                for b in range(4):
                    nc.tensor.transpose(ytp[0:64, b * P:(b + 1) * P],
                                        yn[:, b, :], ident)
                    nc.tensor.transpose(ytp[64:128, b * P:(b + 1) * P],
                                        yn[:, 4 + b, :], ident,
                                        tile_position=(0, 64))
                cols = slice(p * T + 512 * j, p * T + 512 * (j + 1))
                nc.vector.tensor_copy(yT_sb[:, cols], ytp)

            return drains, tail0, tail1a, tail1b

        def outproj_mm(u, op_ps, pairs):
            for pair in pairs:
                nc.tensor.matmul(op_ps,
                                 yT_sb[:, pair * T + u * P: pair * T + (u + 1) * P],
                                 wp_sb[:, pair, :], start=(pair == 0),
                                 stop=(pair == 3))

        def outproj_out(u, op_ps, copy_eng=None):
            o32 = outp.tile([P, 512], F32, tag="o32")
            if copy_eng is nc.scalar:
                nc.scalar.activation(o32, op_ps, AF.Copy)
            else:
                (copy_eng or nc.vector).tensor_copy(o32, op_ps)
            nc.sync.dma_start(out=out[u * P:(u + 1) * P, :], in_=o32)

        def outproj_ttile(u, tag="op", copy_eng=None, pool=None):
            op_ps = (pool or psC).tile([P, 512], F32, tag=tag, bufs=1,
                                       name="op_ps")
            outproj_mm(u, op_ps, range(4))
            outproj_out(u, op_ps, copy_eng)

        for _rep in range(reps):
            # Software-pipelined emission. Prep runs one pair-cycle ahead of
            # need so the A-chain (DVE) latency never blocks attention row
            # transitions. Prologue: A/B interleaved, with B's transposes
            # rotating through the idle attention psum tags.
            preps = {}
            preps[0] = prep_a(0)
            pro_tags = ("yTe", "yTo", "op", "yTe", "yTo")
            for tau in range(1, 5):
                if tau < TT:
                    preps[tau] = prep_a(tau)
                prep_b(tau - 1, preps.pop(tau - 1), tr_tag=pro_tags[tau - 1])
            # wp only needed by the first out-proj, one full row in
            nc.scalar.dma_start(out=wp_sb,
                                in_=wpT.rearrange("(c p) d -> p c d", p=P))
            # Front-load prep emission into rows 0-1 (which have engine
            # slack) so the expensive rows 2-3 run pure attention. A runs
            # one step ahead of B; 2-prep cycles split across two filler
            # points inside the k-loop.
            tails = (None, None, None, None)
            next_a = 5
            for c in range(4 * QC):
                j, p = divmod(c, 4)
                fills = []
                late_fills = []
                if j > 0:
                    # out-proj of the previous row hides under this row's
                    # ACT-bound attention
                    late_fills.append(lambda u=4 * (j - 1) + p:
                                      outproj_ttile(u))
                if next_a < TT and next_a <= 5 + c:
                    def do_a(t=next_a):
                        preps[t] = prep_a(t)
                    fills.append(do_a)
                    next_a += 1
                if 4 + c < TT:
                    fills.append(lambda t=4 + c: prep_b(t, preps.pop(t)))
                if c == 4 * QC - 1:
                    # final chunk: pre-accumulate pairs 0..2 of the first
                    # final-row out-proj into the free pq bank — only the
                    # pair-3 matmul remains after the last tail ("op" stays
                    # free for the final ytp)
                    pre = {}

                    def pre_op():
                        u0 = 4 * (QC - 1)
                        pre[u0] = psA.tile([P, 512], F32, tag="pq",
                                           bufs=1, name="op_ps")
                        outproj_mm(u0, pre[u0], range(3))
                    late_fills.append(pre_op)

                tails = attn_pair_chunk(p, j, fills=fills,
                                        late_fills=late_fills,
                                        prev_tail=tails,
                                        final=(c == 4 * QC - 1),
                                        mask_eng=None)
            drains_f, tail0_f, tail1a_f, tail1b_f = tails
            for d in drains_f:
                d()
            tail0_f()
            tail1a_f()
            tail1b_f()
            # final row drain: finish the two pre-accumulated tiles (one
            # matmul each), run the other two in the freed yq banks; copies
            # spread across engines to parallelize the end-of-kernel path
            u0 = 4 * (QC - 1)
            outproj_mm(u0, pre[u0], range(3, 4))
            outproj_out(u0, pre[u0], nc.scalar)
            op13 = psC.tile([P, 512], F32, tag="yTe", bufs=1, name="op_ps")
            outproj_mm(u0 + 1, op13, range(4))
            op14 = psC.tile([P, 512], F32, tag="yTo", bufs=1, name="op_ps")
            outproj_mm(u0 + 2, op14, range(4))
            outproj_out(u0 + 1, op13, nc.vector)
            op15 = psC.tile([P, 512], F32, tag="op", bufs=1, name="op_ps")
            outproj_mm(u0 + 3, op15, range(4))
            outproj_out(u0 + 2, op14, nc.scalar)
            outproj_out(u0 + 3, op15, nc.vector)

    nc.finalize()
    return nc


_NC_CACHE = {}


def _get_nc(T=2048, reps=1):
    key = (T, reps)
    if key not in _NC_CACHE:
        _NC_CACHE[key] = build_kernel(T=T, reps=reps)
    return _NC_CACHE[key]


def make_host_inputs(x_b, wqkvT, wpT, cosd, sind, trid):
    return dict(xT=np.ascontiguousarray(x_b.T).astype(np.float16),
                wqkvT=wqkvT, wpT=wpT, cosd=cosd, sind=sind, trid=trid)


def make_shared_inputs(Wq, Wk, Wv, Wp, T):
    wqkvT = np.ascontiguousarray(
        np.concatenate([Wq, Wk, Wv], 0).T).astype(np.float16)
    wpT = np.ascontiguousarray(Wp.T).astype(np.float16)
    inv = 1.0 / (ROPE_BASE ** (np.arange(0, 64, 2) / 64))
    f = np.outer(np.arange(T), inv)
    cosd = np.cos(f).astype(np.float16)
    sind = np.sin(f).astype(np.float16)
    trid = (np.arange(128)[None, :] >= np.arange(128)[:, None]).astype(np.float16)
    return wqkvT, wpT, cosd, sind, trid


def kernel(x, Wq, Wk, Wv, Wp, reps=1):
    x = np.asarray(x)
    B, T, C = x.shape
    assert (B, C) == (N_CORES, DIM)
    nc = _get_nc(T=T, reps=reps)
    shared = make_shared_inputs(np.asarray(Wq), np.asarray(Wk),
                                np.asarray(Wv), np.asarray(Wp), T)
    in_maps = [make_host_inputs(x[b], *shared) for b in range(B)]
    res = run_bass_kernel_spmd(nc, in_maps, list(range(N_CORES)))
    return np.stack([res.results[b]["out"] for b in range(B)]).astype(np.float32)



# revision 139
# speedup vs baseline: 1.0100x; 1.0100x over previous
"""nn_AttnA: fused QKV-proj + RMSnorm + RoPE + causal GQA attention + out-proj.

Data-parallel over the batch: core b computes batch element b (B=8 = 8 cores,
no collectives). Host pre-transposes/casts weights and x once; the device
kernel is fully self-contained per core.

Device pipeline per core (T=2048, C=512, 8 q-heads / 4 kv-heads, hd=64):
  1. QKV: fp16 matmuls, xT c-tiles stationary, fused [q|k|v] rhs -> a psum
     slot of the shared sc rotation ([128,1024] f32 x2)
  2. RMS stats + rstd (ACT Ln/Exp from table set 6, loaded once) + RoPE on
     DVE; v gets a 65th all-ones column per kv-head so attnV emits softmax
     denominators for free
  3. PE transposes -> qT [d,t] per head pair; kT duplicated into both row
     halves so the pair's score matmuls row-pack (concurrent K=64 strips)
  4. per (head-pair, 512-wide q chunk): both heads' score matmuls fill one
     [128,1024] sc slot; ONE merged ACT Exp (scale=1/8) -> fp16 pT
     [128,1024]; one 3D-strided triangle mask on diagonal blocks; FLIPPED
     attnV: each 128-wide pT q-block is the STATIONARY operand and the
     65-wide v the moving one, so each matmul costs only 65 output columns
     (29.5us PE total instead of 58us) and the softmax denominator lands
     per-PARTITION at out col 64 of bank-sized [128, 4, 128] f32
     accumulators (start=True only on the bank's first matmul: a start
     zeroes a whole 2KB ZERO_REGION). Normalize is then 2 tiny reciprocals
     + 2 per-partition muls -> y [q,d] fp16, transposed back to yT [d,t] by
     8 PE transposes through the op bank. No cross-partition broadcast, no
     partition-shift DMA.
  5. out-proj: yT t-slices stationary x WpT -> [t, o] fp32 -> DRAM

Emission is software-pipelined: prep A runs one chunk ahead of prep B;
attnV trails the score/exp stream by 4-5 k-steps; each chunk's ENTIRE tail
(trailing attnV drain at k=0/1, reciprocals at k=2, normalize muls at k=3,
transposes+writeback at k=5) is threaded into the next chunk's k-loop so
the in-order PE issues next-chunk scores right at the boundary; prep fills
go one-per-k-step from k=3 (qkv psum release via ACT copies for tau<9,
DVE for later taus, matching which engine has queue slack); out-proj fills
wait until k>=8 (their yT is written by the deferred tail). The final-row
out-projs pre-accumulate pairs 0-2 inside the last chunk.

Constraints learned on HW: GPSIMD/Pool cannot touch PSUM; TensorTensor may
read at most ONE input from PSUM; psum start=True zeroes 2KB regions; fp8
(e4m3) DoubleRow fails the 2e-2 gate everywhere (3-6e-2 measured).

Cost-model timeline: 221us (prev session 253us, stub 379us), verified on
HW at rel err 4.6e-4. Engine busy: ACT ~168us (exp floor: 139k softmax
cols at 1.2GHz + 185ns/instr), PE ~149us, DVE ~114us.
"""
import numpy as np
from contextlib import ExitStack

import concourse.bacc as bacc
import concourse.bass as bass
import concourse.tile as tile
from concourse import mybir
from concourse.bass_utils import run_bass_kernel_spmd
from concourse.masks import make_identity

F32 = mybir.dt.float32
F16 = mybir.dt.float16
AF = mybir.ActivationFunctionType

DIM = 512
EPS = 1.1920928955078125e-07
SCALE = 0.125  # 1/sqrt(64)
ROPE_BASE = 10000.0
N_CORES = 8
ACT_SET_LN_EXP = 6  # natural_log_exp_and_others: serves ln + exp + copy


def build_kernel(T=2048, reps=1, variant="full"):
    """reps>1 re-emits the compute body for delta-timing benchmarks."""
    P = 128
    TT = T // 128
    QC = T // 512
    NPAIR = 4
    VW = 65  # v columns per kv-head incl the ones column

    nc = bacc.Bacc()
    xT = nc.declare_dram_parameter("xT", [DIM, T], F16, isOutput=False)
    wqkvT = nc.declare_dram_parameter("wqkvT", [DIM, 1024], F16, isOutput=False)
    wpT = nc.declare_dram_parameter("wpT", [DIM, DIM], F16, isOutput=False)
    cosd = nc.declare_dram_parameter("cosd", [T, 32], F16, isOutput=False)
    sind = nc.declare_dram_parameter("sind", [T, 32], F16, isOutput=False)
    trid = nc.declare_dram_parameter("trid", [P, P], F16, isOutput=False)
    out = nc.declare_dram_parameter("out", [T, DIM], F32, isOutput=True)

    with tile.TileContext(nc) as tc, ExitStack() as ctx:
        consts = ctx.enter_context(tc.tile_pool(name="consts", bufs=1))
        big = ctx.enter_context(tc.tile_pool(name="big", bufs=1))
        work = ctx.enter_context(tc.tile_pool(name="work", bufs=2))
        pT_pool = ctx.enter_context(tc.tile_pool(name="pT", bufs=2))
        outp = ctx.enter_context(tc.tile_pool(name="outp", bufs=4))
        psA = ctx.enter_context(tc.tile_pool(name="psA", bufs=1, space="PSUM"))
        psC = ctx.enter_context(tc.tile_pool(name="psC", bufs=1, space="PSUM"))

        # Single activation-table load serving Ln + Exp + Copy; without it the
        # auto-pass alternates set 5 (ln) / set 0 (exp) at 1283ns per load.
        nc.scalar.add_instruction(mybir.InstLoadActFuncSet(
            name=nc.get_next_instruction_name(),
            act_func_set_id=ACT_SET_LN_EXP, ins=[], outs=[]))

        ident = consts.tile([P, P], F16)
        make_identity(nc, ident)
        eps_b = consts.tile([P, 1], F32)
        nc.vector.memset(eps_b, EPS)
        tri = consts.tile([P, P], F16)
        cos_sb = consts.tile([P, TT * 32], F16)
        sin_sb = consts.tile([P, TT * 32], F16)

        xT_sb = big.tile([P, 4, T], F16)
        wqkv_sb = big.tile([P, 4, 1024], F16)
        wp_sb = big.tile([P, 4, DIM], F16)
        # Balance input loads across the two HW DGE queues (SP via nc.sync,
        # ACT via nc.scalar) and merge c-slices into single DMAs — each
        # dma_start costs >1.2us of sequencer issue time, which dominates
        # the prologue if the loads are issued one slice at a time.
        # The DMA transfers serialize on the DMA engine, so order by first
        # use: rope tables, then the xT columns the 5 prologue preps read,
        # then weights, then the rest of xT (consumed from tau 5 on, ~25us
        # in). Issue cost is >1.2us per dma_start, so slices are merged.
        FC = min(4 * P, T)  # xT columns needed by the prologue preps
        nc.sync.dma_start(
            out=xT_sb[:, :, 0:FC],
            in_=xT.rearrange("(c p) t -> p c t", p=P)[:, :, 0:FC])
        nc.scalar.dma_start(out=wqkv_sb[:, 0:2, :],
                            in_=wqkvT[0:2 * P, :].rearrange("(c p) t -> p c t", p=P))
        nc.sync.dma_start(out=wqkv_sb[:, 2:4, :],
                          in_=wqkvT[2 * P:4 * P, :].rearrange("(c p) t -> p c t", p=P))
        nc.scalar.dma_start(out=cos_sb.rearrange("p (tau i) -> p tau i", i=32),
                            in_=cosd.rearrange("(tau p) i -> p tau i", p=P))
        nc.sync.dma_start(out=sin_sb.rearrange("p (tau i) -> p tau i", i=32),
                          in_=sind.rearrange("(tau p) i -> p tau i", p=P))
        if FC < T:
            nc.scalar.dma_start(
                out=xT_sb[:, :, FC:T],
                in_=xT.rearrange("(c p) t -> p c t", p=P)[:, :, FC:T])
        nc.scalar.dma_start(out=tri, in_=trid[:, :])

        # PE p-state warm-up: the cost model runs the PE at half clock until
        # 3us of continuous busy; a transpose train during the input-DMA wait
        # ramps it to 2.4GHz before the first real matmul
        warm_ps = psC.tile([P, P], F16, tag="op", name="warm_ps")
        for _ in range(16):
            nc.tensor.transpose(warm_ps, ident, ident)
        qT_sb = big.tile([P, NPAIR * T], F16)
        kT_sb = big.tile([P, NPAIR * T], F16)
        v_sb = big.tile([P, TT, 4, VW], F16)
        yT_sb = big.tile([P, NPAIR * T], F16)
        # ones column (col 64 of each kv-head group), written once
        nc.vector.memset(v_sb[:, :, :, 64:65], 1.0)

        def prep_a(tau):
            """QKV matmuls + psum->sbuf copies + RMS stats + RoPE -> 'prep'.
            The qkv psum comes from the shared sc rotation."""
            qkv_ps = psA.tile([P, 1024], F32, tag="sc", bufs=2, name="qkv_ps")
            for c in range(4):
                lhs = xT_sb[:, c, tau * P:(tau + 1) * P]
                nc.tensor.matmul(qkv_ps[:, 0:512], lhs, wqkv_sb[:, c, 0:512],
                                 start=(c == 0), stop=(c == 3))
                nc.tensor.matmul(qkv_ps[:, 512:1024], lhs, wqkv_sb[:, c, 512:1024],
                                 start=(c == 0), stop=(c == 3))
            qk16 = work.tile([P, 768], F16, tag="qk16")
            if tau < 9:
                # early: ACT has slack and releases the qkv psum slot fast
                nc.scalar.activation(qk16, qkv_ps[:, 0:768], AF.Copy)
                nc.scalar.activation(
                    v_sb[:, tau, :, 0:64],
                    qkv_ps[:, 768:1024].rearrange("p (h d) -> p h d", d=64),
                    AF.Copy)
            else:
                # late: ACT is the exp-saturated bottleneck; DVE has slack
                # there (GPSIMD cannot touch PSUM on HW)
                nc.vector.tensor_copy(qk16, qkv_ps[:, 0:768])
                nc.vector.tensor_copy(
                    v_sb[:, tau, :, 0:64],
                    qkv_ps[:, 768:1024].rearrange("p (h d) -> p h d", d=64))
            sq16 = work.tile([P, 768], F16, tag="sq16")
            if tau <= 4:
                # DVE is the prep-chain rate limiter while preps overlap the
                # short early rows; ACT has slack there
                nc.scalar.activation(sq16, qk16, AF.Square)
            else:
                nc.vector.tensor_mul(sq16, qk16, qk16)
            ms = work.tile([P, 12], F16, tag="ms")
            with nc.allow_low_precision(reason="fp16 mean-of-squares"):
                nc.vector.tensor_reduce(ms, sq16.rearrange("p (h d) -> p h d", d=64),
                                        axis=mybir.AxisListType.X,
                                        op=mybir.AluOpType.add)
            lns = work.tile([P, 12], F32, tag="lns")
            nc.scalar.activation(lns, ms, AF.Ln, scale=1.0 / 64, bias=eps_b)
            r32 = work.tile([P, 12], F32, tag="r32")
            nc.scalar.activation(r32, lns, AF.Exp, scale=-0.5)
            qkr = work.tile([P, 768], F16, tag="qkr")
            nc.vector.tensor_mul(qkr.rearrange("p (h d) -> p h d", d=64),
                                 qk16.rearrange("p (h d) -> p h d", d=64),
                                 r32[:, :, None].broadcast_to([P, 12, 64]))
            qkrh = qkr.rearrange("p (h d) -> p h d", d=64)
            x1, x2 = qkrh[:, :, 0:32], qkrh[:, :, 32:64]
            c_b = cos_sb[:, tau * 32:(tau + 1) * 32][:, None, :].broadcast_to([P, 12, 32])
            s_b = sin_sb[:, tau * 32:(tau + 1) * 32][:, None, :].broadcast_to([P, 12, 32])
            t1 = work.tile([P, 12, 32], F16, tag="t1")
            t2 = work.tile([P, 12, 32], F16, tag="t2")
            t3 = work.tile([P, 12, 32], F16, tag="t3")
            t4 = work.tile([P, 12, 32], F16, tag="t4")
            nc.vector.tensor_mul(t1, x1, c_b)
            nc.vector.tensor_mul(t2, x2, s_b)
            nc.vector.tensor_mul(t3, x1, s_b)
            nc.vector.tensor_mul(t4, x2, c_b)
            prep = work.tile([P, 768], F16, tag="prep")
            ph = prep.rearrange("p (h d) -> p h d", d=64)
            nc.vector.tensor_add(ph[:, :, 0:32], t1, t2)
            nc.vector.tensor_sub(ph[:, :, 32:64], t4, t3)
            return prep

        def prep_b(tau, prep, tr_tag="pq"):
            """PE transposes of 'prep' + writeback into qT/kT column layout.
            q transposes fill cols 0:512, k (duplicated row halves) 512:1024
            of one [128,1024]-f16 psum bank. During the prologue the attn
            psum tags (psC pool) are free, so transposes rotate through them
            and the pq bank never serializes consecutive prep chains."""
            pool = psA if tr_tag == "pq" else psC
            trk_ps = pool.tile([P, 1024], F16, tag=tr_tag, bufs=1, name="trk_ps")
            for blk in range(4):
                nc.tensor.transpose(trk_ps[:, blk * P:(blk + 1) * P],
                                    prep[:, blk * P:(blk + 1) * P], ident)
            for kv in range(4):
                kin = prep[:, 512 + kv * 64: 512 + (kv + 1) * 64]
                nc.tensor.transpose(trk_ps[0:64, 512 + kv * P: 512 + (kv + 1) * P],
                                    kin, ident)
                nc.tensor.transpose(trk_ps[64:128, 512 + kv * P: 512 + (kv + 1) * P],
                                    kin, ident, tile_position=(0, 64))
            qdst = bass.AP(tensor=qT_sb.tensor, offset=qT_sb.offset + tau * P,
                           ap=[qT_sb.ap[0], [T, 4], [1, P]])
            kdst = bass.AP(tensor=kT_sb.tensor, offset=kT_sb.offset + tau * P,
                           ap=[kT_sb.ap[0], [T, 4], [1, P]])
            nc.vector.tensor_copy(qdst, trk_ps[:, 0:512].rearrange("p (g t) -> p g t", t=P))
            nc.vector.tensor_copy(kdst, trk_ps[:, 512:1024].rearrange("p (g t) -> p g t", t=P))

        def attn_pair_chunk(p, j, fills=(), late_fills=(),
                            prev_tail=(None, None), final=False,
                            mask_eng=None):
            """Emits one (head-pair, 512-q-chunk) of attention. Returns two
            tail closures (reciprocal; broadcast+normalize+shift) that the
            CALLER threads into the next chunk's k-loop — emitted at k=0/k=1
            there, they overlap the tail latency with the next chunk's score
            stream instead of stalling the in-order PE at the boundary."""
            nkt = 4 * j + 4
            fills = list(fills)
            late_fills = list(late_fills)
            # [q-part, block, d+den] accumulators, one per head (flipped
            # attnV: pT stationary, 65-wide v moving -> 65-col matmuls and
            # per-PARTITION softmax denominators at out col 64).
            # Block stride is 128 f32 so each tile is exactly one 2KB psum
            # bank: start=True zeroes a whole ZERO_REGION (2KB), so only
            # the very first matmul of each bank may carry it — the other
            # blocks' first writes land on pending-zero bytes (write mode).
            yq_e = psC.tile([P, 4, P], F32, tag="yTe", name="yq_e")
            yq_o = psC.tile([P, 4, P], F32, tag="yTo", name="yq_o")
            pTs = {}

            def attn_v(k):
                pT = pTs.pop(k)
                v65 = v_sb[:, k, p, :]
                for b in range(4):
                    if 4 * j + b < k:
                        continue  # q-block entirely above the diagonal
                    st = (k == 0 and b == 0)
                    sp = (k == 4 * j + b)
                    nc.tensor.matmul(yq_e[:, b, 0:VW], pT[:, b * P:(b + 1) * P],
                                     v65, start=st, stop=sp,
                                     skip_group_check=not st)
                    nc.tensor.matmul(yq_o[:, b, 0:VW],
                                     pT[:, 512 + b * P:512 + (b + 1) * P],
                                     v65, start=st, stop=sp,
                                     skip_group_check=not st)

            for k in range(nkt):
                offs = max(0, P * (k - 4 * j))
                kcol = p * T + k * P
                qcol = p * T + 512 * j + offs
                n = 512 - offs
                # both heads' scores fill one [128,1024] slot; 2-deep
                # rotation lets scores-mm(k+1) overlap exp(k)
                sc = psA.tile([P, 1024], F32, tag="sc", bufs=2, name="sc")
                nc.tensor.matmul(sc[:, offs:512],
                                 kT_sb[0:64, kcol:kcol + P],
                                 qT_sb[0:64, qcol:qcol + n],
                                 start=True, stop=True)
                nc.tensor.matmul(sc[:, 512 + offs:1024],
                                 kT_sb[64:128, kcol:kcol + P],
                                 qT_sb[64:128, qcol:qcol + n],
                                 start=True, stop=True, tile_position=(64, 0))
                pT = pT_pool.tile([P, 1024], F16, tag="pT", bufs=8)
                pTs[k] = pT
                sch = sc.rearrange("p (h n) -> p h n", n=512)
                pTh = pT.rearrange("p (h n) -> p h n", n=512)
                # ONE merged exp for both heads (3D strided AP)
                nc.scalar.activation(pTh[:, :, offs:512], sch[:, :, offs:512],
                                     AF.Exp, scale=SCALE)
                if k >= 4 * j:  # diagonal tile: mask strict lower triangle
                    # on the (otherwise idle) Pool engine — SBUF-only op, so
                    # it is legal there and keeps the chain off the DVE queue
                    sl = slice(offs, offs + P)
                    (mask_eng or nc.vector).tensor_mul(
                        pTh[:, :, sl], pTh[:, :, sl],
                        tri[:, None, :].broadcast_to([P, 2, P]))
                # Thread the PREVIOUS chunk's trailing work (attnV drain,
                # recips, normalize) into this chunk's score stream: the
                # in-order PE then issues this chunk's scores right at the
                # boundary instead of serially waiting exp->mask->attnV->
                # normalize of the previous chunk (was a ~2.9us ACT gap
                # per chunk).
                if prev_tail[0] is not None:
                    pdrains, pt0, pt1a, pt1b = prev_tail
                    nd = len(pdrains)
                    if k == 0:
                        for d in pdrains[:nd // 2]:
                            d()
                    elif k == 1:
                        for d in pdrains[nd // 2:]:
                            d()
                    elif k == 2:
                        pt0()
                    elif k == 3:
                        pt1a()
                    elif k == min(5, nkt - 1):
                        pt1b()
                # attnV trails the score/exp stream so the PE never blocks
                # on the exp of the current k
                dly = 4 if nkt <= 8 else 5
                if k >= dly:
                    attn_v(k - dly)
                # Prep emissions fill from k=3, one per k-step so the burst
                # never outruns the 2-deep sc pipeline and starves ACT.
                # Out-proj fills (late_fills) must wait for the k=5 thread
                # point: they read yT columns that the previous chunk's
                # deferred tail1b writes there.
                if k >= 3 and fills:
                    fills.pop(0)()
                elif k >= 9 and late_fills:
                    late_fills.pop(0)()
            if prev_tail[0] is not None and nkt <= 4:
                # nkt=4 chunks: the k=5 thread point doesn't exist
                pt1b()
            for f in fills + late_fills:
                if f is not None:
                    f()
            drains = [lambda k=k: attn_v(k)
                      for k in range(max(0, nkt - dly), nkt)]

            def tail0():
                # per-partition denominators at out col 64 -> tiny recips
                # (divide-from-psum is illegal: only one PSUM input allowed)
                rd = outp.tile([P, 8], F32, tag="rd")
                nc.vector.reciprocal(rd[:, 0:4], yq_e[:, :, 64])
                nc.vector.reciprocal(rd[:, 4:8], yq_o[:, :, 64])
                tail0.rd = rd

            def tail1a():
                rd = tail0.rd
                yn = outp.tile([P, 8, 64], F16, tag="yn")
                nc.vector.tensor_mul(
                    yn[:, 0:4, :], yq_e[:, :, 0:64],
                    rd[:, 0:4][:, :, None].broadcast_to([P, 4, 64]))
                nc.vector.tensor_mul(
                    yn[:, 4:8, :], yq_o[:, :, 0:64],
                    rd[:, 4:8][:, :, None].broadcast_to([P, 4, 64]))
                tail1a.yn = yn

            def tail1b():
                yn = tail1a.yn
                # transpose y [q,d] back to yT [d,q]; odd head packs into
                # partitions 64..127 via tile_position. Runs 2 k-steps after
                # tail1a so the PE never waits on the DVE normalize muls.
                ytp = psC.tile([P, 512], F16, tag="op", name="ytp")
                for b in range(4):
                    nc.tensor.transpose(ytp[0:64, b * P:(b + 1) * P],
                                        yn[:, b, :], ident)
                    nc.tensor.transpose(ytp[64:128, b * P:(b + 1) * P],
                                        yn[:, 4 + b, :], ident,
                                        tile_position=(0, 64))
                cols = slice(p * T + 512 * j, p * T + 512 * (j + 1))
                nc.vector.tensor_copy(yT_sb[:, cols], ytp)

            return drains, tail0, tail1a, tail1b

        def outproj_mm(u, op_ps, pairs):
            for pair in pairs:
                nc.tensor.matmul(op_ps,
                                 yT_sb[:, pair * T + u * P: pair * T + (u + 1) * P],
                                 wp_sb[:, pair, :], start=(pair == 0),
                                 stop=(pair == 3))

        def outproj_out(u, op_ps, copy_eng=None):
            o32 = outp.tile([P, 512], F32, tag="o32")
            if copy_eng is nc.scalar:
                nc.scalar.activation(o32, op_ps, AF.Copy)
            else:
                (copy_eng or nc.vector).tensor_copy(o32, op_ps)
            nc.sync.dma_start(out=out[u * P:(u + 1) * P, :], in_=o32)

        def outproj_ttile(u, tag="op", copy_eng=None, pool=None):
            op_ps = (pool or psC).tile([P, 512], F32, tag=tag, bufs=1,
                                       name="op_ps")
            outproj_mm(u, op_ps, range(4))
            outproj_out(u, op_ps, copy_eng)

        for _rep in range(reps):
            # Software-pipelined emission. Prep runs one pair-cycle ahead of
            # need so the A-chain (DVE) latency never blocks attention row
            # transitions. Prologue: A/B interleaved, with B's transposes
            # rotating through the idle attention psum tags.
            preps = {}
            preps[0] = prep_a(0)
            pro_tags = ("yTe", "yTo", "op", "yTe", "yTo")
            for tau in range(1, 7):
                if tau < TT:
                    preps[tau] = prep_a(tau)
                if tau - 1 < 4:
                    prep_b(tau - 1, preps.pop(tau - 1), tr_tag=pro_tags[tau - 1])
            # wp only needed by the first out-proj, one full row in
            nc.scalar.dma_start(out=wp_sb,
                                in_=wpT.rearrange("(c p) d -> p c d", p=P))
            # Front-load prep emission into rows 0-1 (which have engine
            # slack) so the expensive rows 2-3 run pure attention. A runs
            # one step ahead of B; 2-prep cycles split across two filler
            # points inside the k-loop.
            tails = (None, None, None, None)
            next_a = 7
            for c in range(4 * QC):
                j, p = divmod(c, 4)
                fills = []
                late_fills = []
                if j > 0:
                    # out-proj of the previous row hides under this row's
                    # ACT-bound attention
                    late_fills.append(lambda u=4 * (j - 1) + p:
                                      outproj_ttile(u))
                if next_a < TT and next_a <= 5 + c:
                    def do_a(t=next_a):
                        preps[t] = prep_a(t)
                    fills.append(do_a)
                    next_a += 1
                if 4 + c < TT:
                    fills.append(lambda t=4 + c: prep_b(t, preps.pop(t)))
                if c == 4 * QC - 1:
                    # final chunk: pre-accumulate pairs 0..2 of the first
                    # final-row out-proj into the free pq bank — only the
                    # pair-3 matmul remains after the last tail ("op" stays
                    # free for the final ytp)
                    pre = {}

                    def pre_op():
                        u0 = 4 * (QC - 1)
                        pre[u0] = psA.tile([P, 512], F32, tag="pq",
                                           bufs=1, name="op_ps")
                        outproj_mm(u0, pre[u0], range(3))
                    late_fills.append(pre_op)

                tails = attn_pair_chunk(p, j, fills=fills,
                                        late_fills=late_fills,
                                        prev_tail=tails,
                                        final=(c == 4 * QC - 1),
                                        mask_eng=None)
            drains_f, tail0_f, tail1a_f, tail1b_f = tails
            for d in drains_f:
                d()
            tail0_f()
            tail1a_f()
            tail1b_f()
            # final row drain: finish the two pre-accumulated tiles (one
            # matmul each), run the other two in the freed yq banks; copies
            # spread across engines to parallelize the end-of-kernel path
            u0 = 4 * (QC - 1)
            outproj_mm(u0, pre[u0], range(3, 4))
            outproj_out(u0, pre[u0], nc.scalar)
            op13 = psC.tile([P, 512], F32, tag="yTe", bufs=1, name="op_ps")
            outproj_mm(u0 + 1, op13, range(4))
            op14 = psC.tile([P, 512], F32, tag="yTo", bufs=1, name="op_ps")
            outproj_mm(u0 + 2, op14, range(4))
            outproj_out(u0 + 1, op13, nc.vector)
            op15 = psC.tile([P, 512], F32, tag="op", bufs=1, name="op_ps")
            outproj_mm(u0 + 3, op15, range(4))
            outproj_out(u0 + 2, op14, nc.scalar)
            outproj_out(u0 + 3, op15, nc.vector)

    nc.finalize()
    return nc


_NC_CACHE = {}


def _get_nc(T=2048, reps=1):
    key = (T, reps)
    if key not in _NC_CACHE:
        _NC_CACHE[key] = build_kernel(T=T, reps=reps)
    return _NC_CACHE[key]


def make_host_inputs(x_b, wqkvT, wpT, cosd, sind, trid):
    return dict(xT=np.ascontiguousarray(x_b.T).astype(np.float16),
                wqkvT=wqkvT, wpT=wpT, cosd=cosd, sind=sind, trid=trid)


def make_shared_inputs(Wq, Wk, Wv, Wp, T):
    wqkvT = np.ascontiguousarray(
        np.concatenate([Wq, Wk, Wv], 0).T).astype(np.float16)
    wpT = np.ascontiguousarray(Wp.T).astype(np.float16)
    inv = 1.0 / (ROPE_BASE ** (np.arange(0, 64, 2) / 64))
    f = np.outer(np.arange(T), inv)
    cosd = np.cos(f).astype(np.float16)
    sind = np.sin(f).astype(np.float16)
    trid = (np.arange(128)[None, :] >= np.arange(128)[:, None]).astype(np.float16)
    return wqkvT, wpT, cosd, sind, trid


def kernel(x, Wq, Wk, Wv, Wp, reps=1):
    x = np.asarray(x)
    B, T, C = x.shape
    assert (B, C) == (N_CORES, DIM)
    nc = _get_nc(T=T, reps=reps)
    shared = make_shared_inputs(np.asarray(Wq), np.asarray(Wk),
                                np.asarray(Wv), np.asarray(Wp), T)
    in_maps = [make_host_inputs(x[b], *shared) for b in range(B)]
    res = run_bass_kernel_spmd(nc, in_maps, list(range(N_CORES)))
    return np.stack([res.results[b]["out"] for b in range(B)]).astype(np.float32)



# revision 148
# speedup vs baseline: 1.0134x; 1.0033x over previous
"""nn_AttnA: fused QKV-proj + RMSnorm + RoPE + causal GQA attention + out-proj.

Data-parallel over the batch: core b computes batch element b (B=8 = 8 cores,
no collectives). Host pre-transposes/casts weights and x once; the device
kernel is fully self-contained per core.

Device pipeline per core (T=2048, C=512, 8 q-heads / 4 kv-heads, hd=64):
  1. QKV: fp16 matmuls, xT c-tiles stationary, fused [q|k|v] rhs -> a psum
     slot of the shared sc rotation ([128,1024] f32 x2)
  2. RMS stats + rstd (ACT Ln/Exp from table set 6, loaded once) + RoPE on
     DVE; v gets a 65th all-ones column per kv-head so attnV emits softmax
     denominators for free
  3. PE transposes -> qT [d,t] per head pair; kT duplicated into both row
     halves so the pair's score matmuls row-pack (concurrent K=64 strips)
  4. per (head-pair, 512-wide q chunk): both heads' score matmuls fill one
     [128,1024] sc slot; ONE merged ACT Exp (scale=1/8) -> fp16 pT
     [128,1024]; one 3D-strided triangle mask on diagonal blocks; FLIPPED
     attnV: each 128-wide pT q-block is the STATIONARY operand and the
     65-wide v the moving one, so each matmul costs only 65 output columns
     (29.5us PE total instead of 58us) and the softmax denominator lands
     per-PARTITION at out col 64 of bank-sized [128, 4, 128] f32
     accumulators (start=True only on the bank's first matmul: a start
     zeroes a whole 2KB ZERO_REGION). Normalize is then 2 tiny reciprocals
     + 2 per-partition muls -> y [q,d] fp16, transposed back to yT [d,t] by
     8 PE transposes through the op bank. No cross-partition broadcast, no
     partition-shift DMA.
  5. out-proj: yT t-slices stationary x WpT -> [t, o] fp32 -> DRAM

Emission is software-pipelined: prep A runs one chunk ahead of prep B;
attnV trails the score/exp stream by 4-5 k-steps; each chunk's ENTIRE tail
(trailing attnV drain at k=0/1, reciprocals at k=2, normalize muls at k=3,
transposes+writeback at k=5) is threaded into the next chunk's k-loop so
the in-order PE issues next-chunk scores right at the boundary; prep fills
go one-per-k-step from k=3 (qkv psum release via ACT copies for tau<9,
DVE for later taus, matching which engine has queue slack); out-proj fills
wait until k>=8 (their yT is written by the deferred tail). The final-row
out-projs pre-accumulate pairs 0-2 inside the last chunk.

Constraints learned on HW: GPSIMD/Pool cannot touch PSUM; TensorTensor may
read at most ONE input from PSUM; psum start=True zeroes 2KB regions; fp8
(e4m3) DoubleRow fails the 2e-2 gate everywhere (3-6e-2 measured).

Cost-model timeline: 219us (prev session 253us, stub 379us), verified on
HW at rel err 4.6e-4. Engine busy: ACT ~168us (exp floor: 139k softmax
cols at 1.2GHz + 185ns/instr), PE ~149us, DVE ~114us.
"""
import numpy as np
from contextlib import ExitStack

import concourse.bacc as bacc
import concourse.bass as bass
import concourse.tile as tile
from concourse import mybir
from concourse.bass_utils import run_bass_kernel_spmd
from concourse.masks import make_identity

F32 = mybir.dt.float32
F16 = mybir.dt.float16
AF = mybir.ActivationFunctionType

DIM = 512
EPS = 1.1920928955078125e-07
SCALE = 0.125  # 1/sqrt(64)
ROPE_BASE = 10000.0
N_CORES = 8
ACT_SET_LN_EXP = 6  # natural_log_exp_and_others: serves ln + exp + copy


def build_kernel(T=2048, reps=1, variant="full"):
    """reps>1 re-emits the compute body for delta-timing benchmarks."""
    P = 128
    TT = T // 128
    QC = T // 512
    NPAIR = 4
    VW = 65  # v columns per kv-head incl the ones column

    nc = bacc.Bacc()
    xT = nc.declare_dram_parameter("xT", [DIM, T], F16, isOutput=False)
    wqkvT = nc.declare_dram_parameter("wqkvT", [DIM, 1024], F16, isOutput=False)
    wpT = nc.declare_dram_parameter("wpT", [DIM, DIM], F16, isOutput=False)
    cosd = nc.declare_dram_parameter("cosd", [T, 32], F16, isOutput=False)
    sind = nc.declare_dram_parameter("sind", [T, 32], F16, isOutput=False)
    trid = nc.declare_dram_parameter("trid", [P, P], F16, isOutput=False)
    out = nc.declare_dram_parameter("out", [T, DIM], F32, isOutput=True)

    with tile.TileContext(nc) as tc, ExitStack() as ctx:
        consts = ctx.enter_context(tc.tile_pool(name="consts", bufs=1))
        big = ctx.enter_context(tc.tile_pool(name="big", bufs=1))
        work = ctx.enter_context(tc.tile_pool(name="work", bufs=2))
        pT_pool = ctx.enter_context(tc.tile_pool(name="pT", bufs=2))
        outp = ctx.enter_context(tc.tile_pool(name="outp", bufs=4))
        psA = ctx.enter_context(tc.tile_pool(name="psA", bufs=1, space="PSUM"))
        psC = ctx.enter_context(tc.tile_pool(name="psC", bufs=1, space="PSUM"))

        # Single activation-table load serving Ln + Exp + Copy; without it the
        # auto-pass alternates set 5 (ln) / set 0 (exp) at 1283ns per load.
        nc.scalar.add_instruction(mybir.InstLoadActFuncSet(
            name=nc.get_next_instruction_name(),
            act_func_set_id=ACT_SET_LN_EXP, ins=[], outs=[]))

        ident = consts.tile([P, P], F16)
        make_identity(nc, ident)
        eps_b = consts.tile([P, 1], F32)
        nc.vector.memset(eps_b, EPS)
        tri = consts.tile([P, P], F16)
        cos_sb = consts.tile([P, TT * 32], F16)
        sin_sb = consts.tile([P, TT * 32], F16)

        xT_sb = big.tile([P, 4, T], F16)
        wqkv_sb = big.tile([P, 4, 1024], F16)
        wp_sb = big.tile([P, 4, DIM], F16)
        # Balance input loads across the two HW DGE queues (SP via nc.sync,
        # ACT via nc.scalar) and merge c-slices into single DMAs — each
        # dma_start costs >1.2us of sequencer issue time, which dominates
        # the prologue if the loads are issued one slice at a time.
        # The DMA transfers serialize on the DMA engine, so order by first
        # use: rope tables, then the xT columns the 5 prologue preps read,
        # then weights, then the rest of xT (consumed from tau 5 on, ~25us
        # in). Issue cost is >1.2us per dma_start, so slices are merged.
        FC = min(4 * P, T)  # xT columns needed by the prologue preps
        nc.sync.dma_start(
            out=xT_sb[:, :, 0:FC],
            in_=xT.rearrange("(c p) t -> p c t", p=P)[:, :, 0:FC])
        nc.scalar.dma_start(out=wqkv_sb[:, 0:2, :],
                            in_=wqkvT[0:2 * P, :].rearrange("(c p) t -> p c t", p=P))
        nc.sync.dma_start(out=wqkv_sb[:, 2:4, :],
                          in_=wqkvT[2 * P:4 * P, :].rearrange("(c p) t -> p c t", p=P))
        nc.scalar.dma_start(out=cos_sb.rearrange("p (tau i) -> p tau i", i=32),
                            in_=cosd.rearrange("(tau p) i -> p tau i", p=P))
        nc.sync.dma_start(out=sin_sb.rearrange("p (tau i) -> p tau i", i=32),
                          in_=sind.rearrange("(tau p) i -> p tau i", p=P))
        if FC < T:
            nc.scalar.dma_start(
                out=xT_sb[:, :, FC:T],
                in_=xT.rearrange("(c p) t -> p c t", p=P)[:, :, FC:T])
        nc.scalar.dma_start(out=tri, in_=trid[:, :])

        # PE p-state warm-up: the cost model runs the PE at half clock until
        # 3us of continuous busy; a transpose train during the input-DMA wait
        # ramps it to 2.4GHz before the first real matmul
        warm_ps = psC.tile([P, P], F16, tag="op", name="warm_ps")
        for _ in range(16):
            nc.tensor.transpose(warm_ps, ident, ident)
        qT_sb = big.tile([P, NPAIR * T], F16)
        kT_sb = big.tile([P, NPAIR * T], F16)
        v_sb = big.tile([P, TT, 4, VW], F16)
        yT_sb = big.tile([P, NPAIR * T], F16)
        # ones column (col 64 of each kv-head group), written once
        nc.vector.memset(v_sb[:, :, :, 64:65], 1.0)

        def prep_a(tau):
            """QKV matmuls + psum->sbuf copies + RMS stats + RoPE -> 'prep'.
            The qkv psum comes from the shared sc rotation."""
            qkv_ps = psA.tile([P, 1024], F32, tag="sc", bufs=2, name="qkv_ps")
            for c in range(4):
                lhs = xT_sb[:, c, tau * P:(tau + 1) * P]
                nc.tensor.matmul(qkv_ps[:, 0:512], lhs, wqkv_sb[:, c, 0:512],
                                 start=(c == 0), stop=(c == 3))
                nc.tensor.matmul(qkv_ps[:, 512:1024], lhs, wqkv_sb[:, c, 512:1024],
                                 start=(c == 0), stop=(c == 3))
            qk16 = work.tile([P, 768], F16, tag="qk16")
            if tau < 9:
                # early: ACT has slack and releases the qkv psum slot fast
                nc.scalar.activation(qk16, qkv_ps[:, 0:768], AF.Copy)
                nc.scalar.activation(
                    v_sb[:, tau, :, 0:64],
                    qkv_ps[:, 768:1024].rearrange("p (h d) -> p h d", d=64),
                    AF.Copy)
            else:
                # late: ACT is the exp-saturated bottleneck; DVE has slack
                # there (GPSIMD cannot touch PSUM on HW)
                nc.vector.tensor_copy(qk16, qkv_ps[:, 0:768])
                nc.vector.tensor_copy(
                    v_sb[:, tau, :, 0:64],
                    qkv_ps[:, 768:1024].rearrange("p (h d) -> p h d", d=64))
            sq16 = work.tile([P, 768], F16, tag="sq16")
            if tau <= 4:
                # DVE is the prep-chain rate limiter while preps overlap the
                # short early rows; ACT has slack there
                nc.scalar.activation(sq16, qk16, AF.Square)
            else:
                nc.vector.tensor_mul(sq16, qk16, qk16)
            ms = work.tile([P, 12], F16, tag="ms")
            with nc.allow_low_precision(reason="fp16 mean-of-squares"):
                nc.vector.tensor_reduce(ms, sq16.rearrange("p (h d) -> p h d", d=64),
                                        axis=mybir.AxisListType.X,
                                        op=mybir.AluOpType.add)
            lns = work.tile([P, 12], F32, tag="lns")
            nc.scalar.activation(lns, ms, AF.Ln, scale=1.0 / 64, bias=eps_b)
            r32 = work.tile([P, 12], F32, tag="r32")
            nc.scalar.activation(r32, lns, AF.Exp, scale=-0.5)
            qkr = work.tile([P, 768], F16, tag="qkr")
            nc.vector.tensor_mul(qkr.rearrange("p (h d) -> p h d", d=64),
                                 qk16.rearrange("p (h d) -> p h d", d=64),
                                 r32[:, :, None].broadcast_to([P, 12, 64]))
            qkrh = qkr.rearrange("p (h d) -> p h d", d=64)
            x1, x2 = qkrh[:, :, 0:32], qkrh[:, :, 32:64]
            c_b = cos_sb[:, tau * 32:(tau + 1) * 32][:, None, :].broadcast_to([P, 12, 32])
            s_b = sin_sb[:, tau * 32:(tau + 1) * 32][:, None, :].broadcast_to([P, 12, 32])
            t1 = work.tile([P, 12, 32], F16, tag="t1")
            t2 = work.tile([P, 12, 32], F16, tag="t2")
            t3 = work.tile([P, 12, 32], F16, tag="t3")
            t4 = work.tile([P, 12, 32], F16, tag="t4")
            nc.vector.tensor_mul(t1, x1, c_b)
            nc.vector.tensor_mul(t2, x2, s_b)
            nc.vector.tensor_mul(t3, x1, s_b)
            nc.vector.tensor_mul(t4, x2, c_b)
            prep = work.tile([P, 768], F16, tag="prep")
            ph = prep.rearrange("p (h d) -> p h d", d=64)
            nc.vector.tensor_add(ph[:, :, 0:32], t1, t2)
            nc.vector.tensor_sub(ph[:, :, 32:64], t4, t3)
            return prep

        def prep_b(tau, prep, tr_tag="pq"):
            """PE transposes of 'prep' + writeback into qT/kT column layout.
            q transposes fill cols 0:512, k (duplicated row halves) 512:1024
            of one [128,1024]-f16 psum bank. During the prologue the attn
            psum tags (psC pool) are free, so transposes rotate through them
            and the pq bank never serializes consecutive prep chains."""
            pool = psA if tr_tag == "pq" else psC
            trk_ps = pool.tile([P, 1024], F16, tag=tr_tag, bufs=1, name="trk_ps")
            for blk in range(4):
                nc.tensor.transpose(trk_ps[:, blk * P:(blk + 1) * P],
                                    prep[:, blk * P:(blk + 1) * P], ident)
            for kv in range(4):
                kin = prep[:, 512 + kv * 64: 512 + (kv + 1) * 64]
                nc.tensor.transpose(trk_ps[0:64, 512 + kv * P: 512 + (kv + 1) * P],
                                    kin, ident)
                nc.tensor.transpose(trk_ps[64:128, 512 + kv * P: 512 + (kv + 1) * P],
                                    kin, ident, tile_position=(0, 64))
            qdst = bass.AP(tensor=qT_sb.tensor, offset=qT_sb.offset + tau * P,
                           ap=[qT_sb.ap[0], [T, 4], [1, P]])
            kdst = bass.AP(tensor=kT_sb.tensor, offset=kT_sb.offset + tau * P,
                           ap=[kT_sb.ap[0], [T, 4], [1, P]])
            nc.vector.tensor_copy(qdst, trk_ps[:, 0:512].rearrange("p (g t) -> p g t", t=P))
            nc.vector.tensor_copy(kdst, trk_ps[:, 512:1024].rearrange("p (g t) -> p g t", t=P))

        def attn_pair_chunk(p, j, fills=(), late_fills=(),
                            prev_tail=(None, None), final=False,
                            mask_eng=None):
            """Emits one (head-pair, 512-q-chunk) of attention. Returns two
            tail closures (reciprocal; broadcast+normalize+shift) that the
            CALLER threads into the next chunk's k-loop — emitted at k=0/k=1
            there, they overlap the tail latency with the next chunk's score
            stream instead of stalling the in-order PE at the boundary."""
            nkt = 4 * j + 4
            fills = list(fills)
            late_fills = list(late_fills)
            # [q-part, block, d+den] accumulators, one per head (flipped
            # attnV: pT stationary, 65-wide v moving -> 65-col matmuls and
            # per-PARTITION softmax denominators at out col 64).
            # Block stride is 128 f32 so each tile is exactly one 2KB psum
            # bank: start=True zeroes a whole ZERO_REGION (2KB), so only
            # the very first matmul of each bank may carry it — the other
            # blocks' first writes land on pending-zero bytes (write mode).
            yq_e = psC.tile([P, 4, P], F32, tag="yTe", name="yq_e")
            yq_o = psC.tile([P, 4, P], F32, tag="yTo", name="yq_o")
            pTs = {}

            def attn_v(k):
                pT = pTs.pop(k)
                v65 = v_sb[:, k, p, :]
                for b in range(4):
                    if 4 * j + b < k:
                        continue  # q-block entirely above the diagonal
                    st = (k == 0 and b == 0)
                    sp = (k == 4 * j + b)
                    nc.tensor.matmul(yq_e[:, b, 0:VW], pT[:, b * P:(b + 1) * P],
                                     v65, start=st, stop=sp,
                                     skip_group_check=not st)
                    nc.tensor.matmul(yq_o[:, b, 0:VW],
                                     pT[:, 512 + b * P:512 + (b + 1) * P],
                                     v65, start=st, stop=sp,
                                     skip_group_check=not st)

            for k in range(nkt):
                offs = max(0, P * (k - 4 * j))
                kcol = p * T + k * P
                qcol = p * T + 512 * j + offs
                n = 512 - offs
                # both heads' scores fill one [128,1024] slot; 2-deep
                # rotation lets scores-mm(k+1) overlap exp(k)
                sc = psA.tile([P, 1024], F32, tag="sc", bufs=2, name="sc")
                nc.tensor.matmul(sc[:, offs:512],
                                 kT_sb[0:64, kcol:kcol + P],
                                 qT_sb[0:64, qcol:qcol + n],
                                 start=True, stop=True)
                nc.tensor.matmul(sc[:, 512 + offs:1024],
                                 kT_sb[64:128, kcol:kcol + P],
                                 qT_sb[64:128, qcol:qcol + n],
                                 start=True, stop=True, tile_position=(64, 0))
                pT = pT_pool.tile([P, 1024], F16, tag="pT", bufs=8)
                pTs[k] = pT
                sch = sc.rearrange("p (h n) -> p h n", n=512)
                pTh = pT.rearrange("p (h n) -> p h n", n=512)
                # ONE merged exp for both heads (3D strided AP)
                nc.scalar.activation(pTh[:, :, offs:512], sch[:, :, offs:512],
                                     AF.Exp, scale=SCALE)
                if k >= 4 * j:  # diagonal tile: mask strict lower triangle
                    # on the (otherwise idle) Pool engine — SBUF-only op, so
                    # it is legal there and keeps the chain off the DVE queue
                    sl = slice(offs, offs + P)
                    (mask_eng or nc.vector).tensor_mul(
                        pTh[:, :, sl], pTh[:, :, sl],
                        tri[:, None, :].broadcast_to([P, 2, P]))
                # Thread the PREVIOUS chunk's trailing work (attnV drain,
                # recips, normalize) into this chunk's score stream: the
                # in-order PE then issues this chunk's scores right at the
                # boundary instead of serially waiting exp->mask->attnV->
                # normalize of the previous chunk (was a ~2.9us ACT gap
                # per chunk).
                if prev_tail[0] is not None:
                    pdrains, pt0, pt1a, pt1b = prev_tail
                    nd = len(pdrains)
                    if k == 0:
                        for d in pdrains[:nd // 2]:
                            d()
                    elif k == 1:
                        for d in pdrains[nd // 2:]:
                            d()
                    elif k == 2:
                        pt0()
                    elif k == 3:
                        pt1a()
                    elif k == min(6, nkt - 1):
                        pt1b()
                # attnV trails the score/exp stream so the PE never blocks
                # on the exp of the current k
                dly = 4 if nkt <= 8 else 5
                if k >= dly:
                    attn_v(k - dly)
                # Prep emissions fill from k=3, one per k-step so the burst
                # never outruns the 2-deep sc pipeline and starves ACT.
                # Out-proj fills (late_fills) must wait for the k=5 thread
                # point: they read yT columns that the previous chunk's
                # deferred tail1b writes there.
                if k >= 3 and fills:
                    fills.pop(0)()
                elif k >= 9 and late_fills:
                    late_fills.pop(0)()
            if prev_tail[0] is not None and nkt <= 4:
                # nkt=4 chunks: the k=5 thread point doesn't exist
                pt1b()
            for f in fills + late_fills:
                if f is not None:
                    f()
            drains = [lambda k=k: attn_v(k)
                      for k in range(max(0, nkt - dly), nkt)]

            def tail0():
                # per-partition denominators at out col 64 -> tiny recips
                # (divide-from-psum is illegal: only one PSUM input allowed)
                rd = outp.tile([P, 8], F32, tag="rd")
                nc.vector.reciprocal(rd[:, 0:4], yq_e[:, :, 64])
                nc.vector.reciprocal(rd[:, 4:8], yq_o[:, :, 64])
                tail0.rd = rd

            def tail1a():
                rd = tail0.rd
                yn = outp.tile([P, 8, 64], F16, tag="yn")
                nc.vector.tensor_mul(
                    yn[:, 0:4, :], yq_e[:, :, 0:64],
                    rd[:, 0:4][:, :, None].broadcast_to([P, 4, 64]))
                nc.vector.tensor_mul(
                    yn[:, 4:8, :], yq_o[:, :, 0:64],
                    rd[:, 4:8][:, :, None].broadcast_to([P, 4, 64]))
                tail1a.yn = yn

            def tail1b():
                yn = tail1a.yn
                # transpose y [q,d] back to yT [d,q]; odd head packs into
                # partitions 64..127 via tile_position. Runs 2 k-steps after
                # tail1a so the PE never waits on the DVE normalize muls.
                ytp = psC.tile([P, 512], F16, tag="op", name="ytp")
                for b in range(4):
                    nc.tensor.transpose(ytp[0:64, b * P:(b + 1) * P],
                                        yn[:, b, :], ident)
                    nc.tensor.transpose(ytp[64:128, b * P:(b + 1) * P],
                                        yn[:, 4 + b, :], ident,
                                        tile_position=(0, 64))
                cols = slice(p * T + 512 * j, p * T + 512 * (j + 1))
                nc.vector.tensor_copy(yT_sb[:, cols], ytp)

            return drains, tail0, tail1a, tail1b

        def outproj_mm(u, op_ps, pairs):
            for pair in pairs:
                nc.tensor.matmul(op_ps,
                                 yT_sb[:, pair * T + u * P: pair * T + (u + 1) * P],
                                 wp_sb[:, pair, :], start=(pair == 0),
                                 stop=(pair == 3))

        def outproj_out(u, op_ps, copy_eng=None):
            o32 = outp.tile([P, 512], F32, tag="o32")
            if copy_eng is nc.scalar:
                nc.scalar.activation(o32, op_ps, AF.Copy)
            else:
                (copy_eng or nc.vector).tensor_copy(o32, op_ps)
            nc.sync.dma_start(out=out[u * P:(u + 1) * P, :], in_=o32)

        def outproj_ttile(u, tag="op", copy_eng=None, pool=None):
            op_ps = (pool or psC).tile([P, 512], F32, tag=tag, bufs=1,
                                       name="op_ps")
            outproj_mm(u, op_ps, range(4))
            outproj_out(u, op_ps, copy_eng)

        for _rep in range(reps):
            # Software-pipelined emission. Prep runs one pair-cycle ahead of
            # need so the A-chain (DVE) latency never blocks attention row
            # transitions. Prologue: A/B interleaved, with B's transposes
            # rotating through the idle attention psum tags.
            preps = {}
            preps[0] = prep_a(0)
            pro_tags = ("yTe", "yTo", "op", "yTe", "yTo")
            for tau in range(1, 7):
                if tau < TT:
                    preps[tau] = prep_a(tau)
                if tau - 1 < 4:
                    prep_b(tau - 1, preps.pop(tau - 1), tr_tag=pro_tags[tau - 1])
            # wp only needed by the first out-proj, one full row in
            nc.scalar.dma_start(out=wp_sb,
                                in_=wpT.rearrange("(c p) d -> p c d", p=P))
            # Front-load prep emission into rows 0-1 (which have engine
            # slack) so the expensive rows 2-3 run pure attention. A runs
            # one step ahead of B; 2-prep cycles split across two filler
            # points inside the k-loop.
            tails = (None, None, None, None)
            next_a = 7
            for c in range(4 * QC):
                j, p = divmod(c, 4)
                fills = []
                late_fills = []
                if j > 0:
                    # out-proj of the previous row hides under this row's
                    # ACT-bound attention
                    late_fills.append(lambda u=4 * (j - 1) + p:
                                      outproj_ttile(u))
                if next_a < TT and next_a <= 5 + c:
                    def do_a(t=next_a):
                        preps[t] = prep_a(t)
                    fills.append(do_a)
                    next_a += 1
                if 4 + c < TT:
                    fills.append(lambda t=4 + c: prep_b(t, preps.pop(t)))
                if c == 4 * QC - 1:
                    # final chunk: pre-accumulate pairs 0..2 of the first
                    # final-row out-proj into the free pq bank — only the
                    # pair-3 matmul remains after the last tail ("op" stays
                    # free for the final ytp)
                    pre = {}

                    def pre_op():
                        u0 = 4 * (QC - 1)
                        pre[u0] = psA.tile([P, 512], F32, tag="pq",
                                           bufs=1, name="op_ps")
                        outproj_mm(u0, pre[u0], range(3))
                    late_fills.append(pre_op)

                tails = attn_pair_chunk(p, j, fills=fills,
                                        late_fills=late_fills,
                                        prev_tail=tails,
                                        final=(c == 4 * QC - 1),
                                        mask_eng=None)
            drains_f, tail0_f, tail1a_f, tail1b_f = tails
            for d in drains_f:
                d()
            tail0_f()
            tail1a_f()
            tail1b_f()
            # final row drain: finish the two pre-accumulated tiles (one
            # matmul each), run the other two in the freed yq banks; copies
            # spread across engines to parallelize the end-of-kernel path
            u0 = 4 * (QC - 1)
            outproj_mm(u0, pre[u0], range(3, 4))
            outproj_out(u0, pre[u0], nc.scalar)
            op13 = psC.tile([P, 512], F32, tag="yTe", bufs=1, name="op_ps")
            outproj_mm(u0 + 1, op13, range(4))
            op14 = psC.tile([P, 512], F32, tag="yTo", bufs=1, name="op_ps")
            outproj_mm(u0 + 2, op14, range(4))
            outproj_out(u0 + 1, op13, nc.vector)
            op15 = psC.tile([P, 512], F32, tag="op", bufs=1, name="op_ps")
            outproj_mm(u0 + 3, op15, range(4))
            outproj_out(u0 + 2, op14, nc.scalar)
            outproj_out(u0 + 3, op15, nc.vector)

    nc.finalize()
    return nc


_NC_CACHE = {}


def _get_nc(T=2048, reps=1):
    key = (T, reps)
    if key not in _NC_CACHE:
        _NC_CACHE[key] = build_kernel(T=T, reps=reps)
    return _NC_CACHE[key]


def make_host_inputs(x_b, wqkvT, wpT, cosd, sind, trid):
    return dict(xT=np.ascontiguousarray(x_b.T).astype(np.float16),
                wqkvT=wqkvT, wpT=wpT, cosd=cosd, sind=sind, trid=trid)


def make_shared_inputs(Wq, Wk, Wv, Wp, T):
    wqkvT = np.ascontiguousarray(
        np.concatenate([Wq, Wk, Wv], 0).T).astype(np.float16)
    wpT = np.ascontiguousarray(Wp.T).astype(np.float16)
    inv = 1.0 / (ROPE_BASE ** (np.arange(0, 64, 2) / 64))
    f = np.outer(np.arange(T), inv)
    cosd = np.cos(f).astype(np.float16)
    sind = np.sin(f).astype(np.float16)
    trid = (np.arange(128)[None, :] >= np.arange(128)[:, None]).astype(np.float16)
    return wqkvT, wpT, cosd, sind, trid


def kernel(x, Wq, Wk, Wv, Wp, reps=1):
    x = np.asarray(x)
    B, T, C = x.shape
    assert (B, C) == (N_CORES, DIM)
    nc = _get_nc(T=T, reps=reps)
    shared = make_shared_inputs(np.asarray(Wq), np.asarray(Wk),
                                np.asarray(Wv), np.asarray(Wp), T)
    in_maps = [make_host_inputs(x[b], *shared) for b in range(B)]
    res = run_bass_kernel_spmd(nc, in_maps, list(range(N_CORES)))
    return np.stack([res.results[b]["out"] for b in range(B)]).astype(np.float32)

